# revision 2
# baseline (speedup 1.0000x reference)
"""Multi-head attention (B=4, S=4096, D=512, H=2) on 8 TRN2 NeuronCores.

Sharding: one (batch, head) pair per core -> 8 cores, perfectly balanced,
no collectives. Host pre-transposes x per batch to x^T (bf16) and slices
the weights per head; device computes the full attention for its pair and
the partial output projection; host sums the two head partials per batch.

Bias handling (exact):
  - bq, bk folded into the PSUM->SBUF copies of Q^T/K^T (per-partition bias).
  - bk is softmax-invariant but folded anyway (exactness for free).
  - bv, bo: softmax rows sum to one, so  norm(P(V+bv))Wo + bo
    = norm(PV)Wo + (bv Wo + bo); the constant row vector is added on host.

Softmax: scores are ~N(0,1) after the 1/sqrt(PD) scaling (|s| < ~7), so
exp() without the max-subtraction is numerically safe in fp32/bf16 and
mathematically identical to jax.nn.softmax after normalization.
"""

import sys
from contextlib import ExitStack

import numpy as np

sys.path.insert(0, "/opt/trn_rl_repo")

import ml_dtypes  # noqa: E402

import concourse.bass as bass  # noqa: E402
import concourse.mybir as mybir  # noqa: E402
import concourse.tile as tile  # noqa: E402
from concourse import bacc  # noqa: E402
from concourse.bass_utils import run_bass_kernel_spmd  # noqa: E402
from concourse.masks import make_identity  # noqa: E402

B, S, D, H = 4, 4096, 512, 2
PD = D // H          # 256 head dim
P = 128              # partitions
CC = D // P          # 4 contraction chunks over D
DT = PD // P         # 2 partition-tiles over head dim
QB = 512             # q block width (PSUM bank)
NQB = S // QB        # 8
NKT = S // P         # 32 k tiles
F32 = mybir.dt.float32
BF16 = mybir.dt.bfloat16
SCALE = 1.0 / float(np.sqrt(PD))
NCORES = 8
AF = mybir.ActivationFunctionType


def _attention_body(tc, out, xT, wq, wk, wv, wo, bq, bk):
    nc = tc.nc
    with ExitStack() as ctx:
        const = ctx.enter_context(tc.tile_pool(name="const", bufs=1))
        qk = ctx.enter_context(tc.tile_pool(name="qk", bufs=1))
        vp = ctx.enter_context(tc.tile_pool(name="vp", bufs=1))
        ptp = ctx.enter_context(tc.tile_pool(name="ptp", bufs=36))
        atp = ctx.enter_context(tc.tile_pool(name="atp", bufs=4))
        smal = ctx.enter_context(tc.tile_pool(name="smal", bufs=4))
        outp = ctx.enter_context(tc.tile_pool(name="outp", bufs=4))
        ps = ctx.enter_context(tc.tile_pool(name="ps", bufs=5, space="PSUM"))
        pst = ctx.enter_context(tc.tile_pool(name="pst", bufs=3, space="PSUM"))

        # constants and weights
        ident = const.tile([P, P], BF16)
        make_identity(nc, ident[:])

        xt_sb = const.tile([P, CC, S], BF16)
        nc.sync.dma_start(out=xt_sb[:], in_=xT.rearrange("(c p) s -> p c s", p=P))
        wq_sb = const.tile([P, CC, PD], BF16)
        nc.sync.dma_start(out=wq_sb[:], in_=wq.rearrange("(c p) d -> p c d", p=P))
        wk_sb = const.tile([P, CC, PD], BF16)
        nc.sync.dma_start(out=wk_sb[:], in_=wk.rearrange("(c p) d -> p c d", p=P))
        wv_sb = const.tile([P, CC, PD], BF16)
        nc.sync.dma_start(out=wv_sb[:], in_=wv.rearrange("(c p) d -> p c d", p=P))
        wo_sb = const.tile([P, DT, D], BF16)
        nc.sync.dma_start(out=wo_sb[:], in_=wo.rearrange("(t p) e -> p t e", p=P))
        bq_sb = const.tile([P, DT], F32)
        nc.sync.dma_start(out=bq_sb[:], in_=bq.rearrange("(t p) -> p t", p=P))
        bk_sb = const.tile([P, DT], F32)
        nc.sync.dma_start(out=bk_sb[:], in_=bk.rearrange("(t p) -> p t", p=P))

        # phase 1: projections
        qt_sb = qk.tile([P, DT, S], BF16)           # Q^T  [d, s]
        kt_sb = qk.tile([P, DT, S], BF16)           # K^T  [d, s]
        v_sb = vp.tile([P, NKT, PD + 1], BF16)      # V    [s, d] + ones col

        for w_sb, b_sb, dst in ((wq_sb, bq_sb, qt_sb), (wk_sb, bk_sb, kt_sb)):
            for dt in range(DT):
                for sb in range(NQB):
                    acc = ps.tile([P, QB], F32, tag="acc")
                    for c in range(CC):
                        nc.tensor.matmul(
                            acc[:],
                            w_sb[:, c, dt * P:(dt + 1) * P],
                            xt_sb[:, c, sb * QB:(sb + 1) * QB],
                            start=(c == 0), stop=(c == CC - 1),
                        )
                    nc.scalar.activation(
                        dst[:, dt, sb * QB:(sb + 1) * QB], acc[:],
                        AF.Identity, bias=b_sb[:, dt:dt + 1],
                    )

        for st in range(NKT):
            acc = ps.tile([P, PD], F32, tag="acc")
            for c in range(CC):
                nc.tensor.matmul(
                    acc[:],
                    xt_sb[:, c, st * P:(st + 1) * P],
                    wv_sb[:, c, :],
                    start=(c == 0), stop=(c == CC - 1),
                )
            nc.vector.tensor_copy(v_sb[:, st, 0:PD], acc[:])
            nc.vector.memset(v_sb[:, st, PD:PD + 1], 1.0)

        # phase 2: attention + output projection, per q block
        for qb in range(NQB):
            q_sl = slice(qb * QB, (qb + 1) * QB)
            # S^T = K^T' Q (k on partitions), exp -> P^T
            pts = []
            for kt in range(NKT):
                acc = ps.tile([P, QB], F32, tag="acc")
                for dt in range(DT):
                    nc.tensor.matmul(
                        acc[:],
                        kt_sb[:, dt, kt * P:(kt + 1) * P],
                        qt_sb[:, dt, q_sl],
                        start=(dt == 0), stop=(dt == DT - 1),
                    )
                ptt = ptp.tile([P, QB], BF16, tag="pt")
                nc.scalar.activation(ptt[:], acc[:], AF.Exp, scale=SCALE)
                pts.append(ptt)

            # PV with ones column (unnormalized attn + row sums)
            att = [
                atp.tile([P, QB], BF16, tag=f"at{dt}", name=f"att{dt}")
                for dt in range(DT)
            ]
            for qt in range(QB // P):
                acc = ps.tile([P, PD + 1], F32, tag="acc")
                for kt in range(NKT):
                    nc.tensor.matmul(
                        acc[:],
                        pts[kt][:, qt * P:(qt + 1) * P],
                        v_sb[:, kt, :],
                        start=(kt == 0), stop=(kt == NKT - 1),
                    )
                rcp = smal.tile([P, 1], F32, tag="rcp")
                nc.vector.reciprocal(rcp[:], acc[:, PD:PD + 1])
                attn_n = smal.tile([P, PD], BF16, tag="attn_n")
                nc.vector.tensor_scalar_mul(attn_n[:], acc[:, 0:PD], rcp[:])
                # transpose to [d, q] for the output projection
                for dt in range(DT):
                    trp = pst.tile([P, P], BF16, tag="tr")
                    nc.tensor.transpose(
                        trp[:], attn_n[:, dt * P:(dt + 1) * P], ident[:]
                    )
                    nc.vector.tensor_copy(att[dt][:, qt * P:(qt + 1) * P], trp[:])

            # O projection: out[s, :] = attn^T.T @ Wo
            for qt in range(QB // P):
                acc = ps.tile([P, D], F32, tag="acc")
                for dt in range(DT):
                    nc.tensor.matmul(
                        acc[:],
                        att[dt][:, qt * P:(qt + 1) * P],
                        wo_sb[:, dt, :],
                        start=(dt == 0), stop=(dt == DT - 1),
                    )
                osb = outp.tile([P, D], F32, tag="out")
                nc.vector.tensor_copy(osb[:], acc[:])
                r0 = qb * QB + qt * P
                nc.sync.dma_start(out=out[r0:r0 + P, :], in_=osb[:])


_NC_CACHE = None


def _build_nc():
    global _NC_CACHE
    if _NC_CACHE is not None:
        return _NC_CACHE
    nc = bacc.Bacc(
        "TRN2", target_bir_lowering=False, debug=False, num_devices=NCORES
    )
    xT = nc.dram_tensor("xT", [D, S], BF16, kind="ExternalInput").ap()
    wq = nc.dram_tensor("wq", [D, PD], BF16, kind="ExternalInput").ap()
    wk = nc.dram_tensor("wk", [D, PD], BF16, kind="ExternalInput").ap()
    wv = nc.dram_tensor("wv", [D, PD], BF16, kind="ExternalInput").ap()
    wo = nc.dram_tensor("wo", [PD, D], BF16, kind="ExternalInput").ap()
    bq = nc.dram_tensor("bq", [PD], F32, kind="ExternalInput").ap()
    bk = nc.dram_tensor("bk", [PD], F32, kind="ExternalInput").ap()
    out = nc.dram_tensor("out", [S, D], F32, kind="ExternalOutput").ap()
    with tile.TileContext(nc) as tc:
        _attention_body(tc, out, xT, wq, wk, wv, wo, bq, bk)
    nc.compile()
    _NC_CACHE = nc
    return nc


def _run(inputs, **spmd_kwargs):
    x = np.asarray(inputs["x"], np.float32)
    Wq = np.asarray(inputs["Wq"], np.float32)
    Wk = np.asarray(inputs["Wk"], np.float32)
    Wv = np.asarray(inputs["Wv"], np.float32)
    Wo = np.asarray(inputs["Wo"], np.float32)
    bq = np.asarray(inputs["bq"], np.float32)
    bk = np.asarray(inputs["bk"], np.float32)
    bv = np.asarray(inputs["bv"], np.float32)
    bo = np.asarray(inputs["bo"], np.float32)

    bf = ml_dtypes.bfloat16
    xT = [np.ascontiguousarray(x[b].T).astype(bf) for b in range(B)]
    in_maps = []
    for core in range(NCORES):
        b, h = divmod(core, H)
        hs = slice(h * PD, (h + 1) * PD)
        in_maps.append({
            "xT": xT[b],
            "wq": np.ascontiguousarray(Wq[:, hs]).astype(bf),
            "wk": np.ascontiguousarray(Wk[:, hs]).astype(bf),
            "wv": np.ascontiguousarray(Wv[:, hs]).astype(bf),
            "wo": np.ascontiguousarray(Wo[hs, :]).astype(bf),
            "bq": np.ascontiguousarray(bq[hs]),
            "bk": np.ascontiguousarray(bk[hs]),
        })

    nc = _build_nc()
    res = run_bass_kernel_spmd(nc, in_maps, list(range(NCORES)), **spmd_kwargs)

    out = np.zeros((B, S, D), np.float32)
    for core in range(NCORES):
        b = core // H
        out[b] += res.results[core]["out"]
    out += bv @ Wo + bo  # exact bias correction (softmax rows sum to 1)
    return out, res


def kernel(**inputs):
    out, _ = _run(inputs)
    return out


# revision 4
# speedup vs baseline: 1.0786x; 1.0786x over previous
"""Multi-head attention (B=4, S=4096, D=512, H=2) on 8 TRN2 NeuronCores.

Sharding: one (batch, head) pair per core -> 8 cores, perfectly balanced,
no collectives. Host pre-transposes x per batch to x^T (bf16) and slices
the weights per head; device computes the full attention for its pair and
the partial output projection; host sums the two head partials per batch.

Bias handling (exact):
  - bq, bk folded into the PSUM->SBUF copies of Q^T/K^T (per-partition bias).
  - bk is softmax-invariant but folded anyway (exactness for free).
  - bv, bo: softmax rows sum to one, so  norm(P(V+bv))Wo + bo
    = norm(PV)Wo + (bv Wo + bo); the constant row vector is added on host.

Softmax: scores are ~N(0,1) after the 1/sqrt(PD) scaling (|s| < ~7), so
exp() without the max-subtraction is numerically safe in fp32/bf16 and
mathematically identical to jax.nn.softmax after normalization.
"""

import sys
from contextlib import ExitStack

import numpy as np

sys.path.insert(0, "/opt/trn_rl_repo")

import ml_dtypes  # noqa: E402

import concourse.bass as bass  # noqa: E402
import concourse.mybir as mybir  # noqa: E402
import concourse.tile as tile  # noqa: E402
from concourse import bacc  # noqa: E402
from concourse.bass_utils import run_bass_kernel_spmd  # noqa: E402
from concourse.masks import make_identity  # noqa: E402

B, S, D, H = 4, 4096, 512, 2
PD = D // H          # 256 head dim
P = 128              # partitions
CC = D // P          # 4 contraction chunks over D
DT = PD // P         # 2 partition-tiles over head dim
QB = 512             # q block width (PSUM bank)
NQB = S // QB        # 8
NKT = S // P         # 32 k tiles
F32 = mybir.dt.float32
BF16 = mybir.dt.bfloat16
SCALE = 1.0 / float(np.sqrt(PD))
NCORES = 8
AF = mybir.ActivationFunctionType


def _attention_body(tc, out, xT, wq, wk, wv, wo, bq, bk):
    nc = tc.nc
    NPAIR = NKT // 2  # 16 S^T pairs per q block (exp over 2 PSUM banks)
    with ExitStack() as ctx:
        const = ctx.enter_context(tc.tile_pool(name="const", bufs=1))
        xtp = ctx.enter_context(tc.tile_pool(name="xtp", bufs=CC))
        qk = ctx.enter_context(tc.tile_pool(name="qk", bufs=1))
        vp = ctx.enter_context(tc.tile_pool(name="vp", bufs=1))
        ptp = ctx.enter_context(tc.tile_pool(name="ptp", bufs=34))
        atp = ctx.enter_context(tc.tile_pool(name="atp", bufs=4))
        smal = ctx.enter_context(tc.tile_pool(name="smal", bufs=6))
        outp = ctx.enter_context(tc.tile_pool(name="outp", bufs=4))
        pstp = ctx.enter_context(tc.tile_pool(name="pstp", bufs=2, space="PSUM"))
        psa = ctx.enter_context(tc.tile_pool(name="psa", bufs=2, space="PSUM"))
        pstr = ctx.enter_context(tc.tile_pool(name="pstr", bufs=2, space="PSUM"))

        # constants and weights (small DMAs first, then x chunks)
        ident = const.tile([P, P], BF16)
        make_identity(nc, ident[:])

        wq_sb = const.tile([P, CC, PD], BF16)
        nc.sync.dma_start(out=wq_sb[:], in_=wq.rearrange("(c p) d -> p c d", p=P))
        wk_sb = const.tile([P, CC, PD], BF16)
        nc.sync.dma_start(out=wk_sb[:], in_=wk.rearrange("(c p) d -> p c d", p=P))
        wv_sb = const.tile([P, CC, PD], BF16)
        nc.sync.dma_start(out=wv_sb[:], in_=wv.rearrange("(c p) d -> p c d", p=P))
        wo_sb = const.tile([P, DT, D], BF16)
        nc.sync.dma_start(out=wo_sb[:], in_=wo.rearrange("(t p) e -> p t e", p=P))
        bq_sb = const.tile([P, DT], F32)
        nc.sync.dma_start(out=bq_sb[:], in_=bq.rearrange("(t p) -> p t", p=P))
        bk_sb = const.tile([P, DT], F32)
        nc.sync.dma_start(out=bk_sb[:], in_=bk.rearrange("(t p) -> p t", p=P))

        xr = xT.rearrange("(c p) s -> c p s", p=P)
        xt_sb = []
        for c in range(CC):
            xc = xtp.tile([P, S], BF16, tag="xt", name=f"xt{c}")
            nc.sync.dma_start(out=xc[:], in_=xr[c])
            xt_sb.append(xc)

        qt_sb = qk.tile([P, DT, S], BF16)           # Q^T  [d, s]
        kt_sb = qk.tile([P, DT, S], BF16)           # K^T  [d, s]
        v_sb = vp.tile([P, NKT, PD + 1], BF16)      # V    [s, d] + ones col
        nc.vector.memset(v_sb[:, :, PD:PD + 1], 1.0)

        def proj_qk(w_sb, b_sb, dst, dt, sb):
            acc = psa.tile([P, QB], F32, tag="acc", name="acc_p")
            for c in range(CC):
                nc.tensor.matmul(
                    acc[:],
                    w_sb[:, c, dt * P:(dt + 1) * P],
                    xt_sb[c][:, sb * QB:(sb + 1) * QB],
                    start=(c == 0), stop=(c == CC - 1),
                )
            nc.vector.tensor_scalar_add(
                dst[:, dt, sb * QB:(sb + 1) * QB], acc[:], b_sb[:, dt:dt + 1]
            )

        def proj_v(st):
            acc = psa.tile([P, PD], F32, tag="acc", name="acc_v")
            for c in range(CC):
                nc.tensor.matmul(
                    acc[:],
                    xt_sb[c][:, st * P:(st + 1) * P],
                    wv_sb[:, c, :],
                    start=(c == 0), stop=(c == CC - 1),
                )
            nc.vector.tensor_copy(v_sb[:, st, 0:PD], acc[:])

        pt_tiles = {}  # (qb, pair) -> tile [P, 2, QB]

        def st_pair(qb, pair):
            # scores^T for k tiles (2*pair, 2*pair+1), exp over both banks
            acc = pstp.tile([P, 2, QB], F32, tag="st", name="acc_st")
            for par in range(2):
                kt = 2 * pair + par
                for dt in range(DT):
                    nc.tensor.matmul(
                        acc[:, par, :],
                        kt_sb[:, dt, kt * P:(kt + 1) * P],
                        qt_sb[:, dt, qb * QB:(qb + 1) * QB],
                        start=(dt == 0), stop=(dt == DT - 1),
                    )
            ptt = ptp.tile([P, 2, QB], BF16, tag="pt", name="ptt")
            nc.scalar.activation(ptt[:], acc[:], AF.Exp, scale=SCALE)
            pt_tiles[(qb, pair)] = ptt

        # interleaved schedule state
        pend = {}

        def at_step(gs, fn):
            pend.setdefault(gs, []).append(fn)

        def flush(gs):
            for fn in pend.pop(gs, []):
                fn()

        att = {}      # (qb, dt) -> attn^T tile [P, QB]
        attn_n = {}   # (qb, qt) -> normalized attn [P, PD]

        def norm(qb, qt, acc):
            rcp = smal.tile([P, 1], F32, tag="rcp", name="rcp")
            nc.vector.reciprocal(rcp[:], acc[:, PD:PD + 1])
            an = smal.tile([P, PD], BF16, tag="attn_n", name="attn_n")
            nc.vector.tensor_scalar_mul(an[:], acc[:, 0:PD], rcp[:])
            attn_n[(qb, qt)] = an

        def tr(qb, qt):
            an = attn_n.pop((qb, qt))
            for dt in range(DT):
                trp = pstr.tile([P, P], BF16, tag="tr", name="trp")
                nc.tensor.transpose(trp[:], an[:, dt * P:(dt + 1) * P], ident[:])
                nc.vector.tensor_copy(
                    att[(qb, dt)][:, qt * P:(qt + 1) * P], trp[:]
                )

        def o_proj(qb, qt):
            acc = psa.tile([P, D], F32, tag="acc", name="acc_o")
            for dt in range(DT):
                nc.tensor.matmul(
                    acc[:],
                    att[(qb, dt)][:, qt * P:(qt + 1) * P],
                    wo_sb[:, dt, :],
                    start=(dt == 0), stop=(dt == DT - 1),
                )
            osb = outp.tile([P, D], F32, tag="out", name="osb")
            nc.vector.tensor_copy(osb[:], acc[:])
            r0 = qb * QB + qt * P
            nc.sync.dma_start(out=out[r0:r0 + P, :], in_=osb[:])

        # ---- prologue: Q(sb0), all K, then V + remaining Q + S^T(0) ----
        for dt in range(DT):
            proj_qk(wq_sb, bq_sb, qt_sb, dt, 0)
        for dt in range(DT):
            for sb in range(NQB):
                proj_qk(wk_sb, bk_sb, kt_sb, dt, sb)
        q_rest = [(dt, sb) for sb in range(1, NQB) for dt in range(DT)]
        for i in range(NKT):
            proj_v(i)
            if i % 2 == 0:
                st_pair(0, i // 2)
            if i % 2 == 1 and q_rest:
                dt, sb = q_rest.pop(0)
                proj_qk(wq_sb, bq_sb, qt_sb, dt, sb)

        # ---- main loop: interleave S^T(qb+1) with PV/norm/TR/O of qb ----
        for qb in range(NQB):
            for d in range(DT):
                att[(qb, d)] = atp.tile([P, QB], BF16, tag=f"at{d}",
                                        name=f"att{d}")
            for step in range(32):
                gs = qb * 32 + step
                qt, j = divmod(step, 8)
                if qb + 1 < NQB and step % 2 == 0:
                    st_pair(qb + 1, step // 2)
                if j == 0:
                    acc_pv = psa.tile([P, PD + 1], F32, tag="acc",
                                      name="acc_pv")
                for m in range(4):
                    kt = j * 4 + m
                    pair, par = divmod(kt, 2)
                    nc.tensor.matmul(
                        acc_pv[:],
                        pt_tiles[(qb, pair)][:, par, qt * P:(qt + 1) * P],
                        v_sb[:, kt, :],
                        start=(kt == 0), stop=(kt == NKT - 1),
                    )
                if j == 7:
                    norm(qb, qt, acc_pv)
                    at_step(gs + 2, lambda qb=qb, qt=qt: tr(qb, qt))
                    at_step(gs + 4, lambda qb=qb, qt=qt: o_proj(qb, qt))
                flush(gs)
            # drop references to consumed P^T tiles of this qb
            for pair in range(NPAIR):
                pt_tiles.pop((qb, pair), None)

        # tail: flush any remaining deferred work (TR/O of the last q tiles)
        for gs in sorted(pend):
            for fn in pend.pop(gs, []):
                fn()


_NC_CACHE = None


def _build_nc():
    global _NC_CACHE
    if _NC_CACHE is not None:
        return _NC_CACHE
    nc = bacc.Bacc(
        "TRN2", target_bir_lowering=False, debug=False, num_devices=NCORES
    )
    xT = nc.dram_tensor("xT", [D, S], BF16, kind="ExternalInput").ap()
    wq = nc.dram_tensor("wq", [D, PD], BF16, kind="ExternalInput").ap()
    wk = nc.dram_tensor("wk", [D, PD], BF16, kind="ExternalInput").ap()
    wv = nc.dram_tensor("wv", [D, PD], BF16, kind="ExternalInput").ap()
    wo = nc.dram_tensor("wo", [PD, D], BF16, kind="ExternalInput").ap()
    bq = nc.dram_tensor("bq", [PD], F32, kind="ExternalInput").ap()
    bk = nc.dram_tensor("bk", [PD], F32, kind="ExternalInput").ap()
    out = nc.dram_tensor("out", [S, D], F32, kind="ExternalOutput").ap()
    with tile.TileContext(nc) as tc:
        _attention_body(tc, out, xT, wq, wk, wv, wo, bq, bk)
    nc.compile()
    _NC_CACHE = nc
    return nc


def _run(inputs, **spmd_kwargs):
    x = np.asarray(inputs["x"], np.float32)
    Wq = np.asarray(inputs["Wq"], np.float32)
    Wk = np.asarray(inputs["Wk"], np.float32)
    Wv = np.asarray(inputs["Wv"], np.float32)
    Wo = np.asarray(inputs["Wo"], np.float32)
    bq = np.asarray(inputs["bq"], np.float32)
    bk = np.asarray(inputs["bk"], np.float32)
    bv = np.asarray(inputs["bv"], np.float32)
    bo = np.asarray(inputs["bo"], np.float32)

    bf = ml_dtypes.bfloat16
    xT = [np.ascontiguousarray(x[b].T).astype(bf) for b in range(B)]
    in_maps = []
    for core in range(NCORES):
        b, h = divmod(core, H)
        hs = slice(h * PD, (h + 1) * PD)
        in_maps.append({
            "xT": xT[b],
            "wq": np.ascontiguousarray(Wq[:, hs]).astype(bf),
            "wk": np.ascontiguousarray(Wk[:, hs]).astype(bf),
            "wv": np.ascontiguousarray(Wv[:, hs]).astype(bf),
            "wo": np.ascontiguousarray(Wo[hs, :]).astype(bf),
            "bq": np.ascontiguousarray(bq[hs]),
            "bk": np.ascontiguousarray(bk[hs]),
        })

    nc = _build_nc()
    res = run_bass_kernel_spmd(nc, in_maps, list(range(NCORES)), **spmd_kwargs)

    out = np.zeros((B, S, D), np.float32)
    for core in range(NCORES):
        b = core // H
        out[b] += res.results[core]["out"]
    out += bv @ Wo + bo  # exact bias correction (softmax rows sum to 1)
    return out, res


def kernel(**inputs):
    out, _ = _run(inputs)
    return out


# revision 7
# speedup vs baseline: 1.1084x; 1.0277x over previous
"""Multi-head attention (B=4, S=4096, D=512, H=2) on 8 TRN2 NeuronCores.

Sharding: one (batch, head) pair per core -> 8 cores, perfectly balanced,
no collectives. Host pre-transposes x per batch to x^T (bf16) and slices
the weights per head; device computes the full attention for its pair and
the partial output projection; host sums the two head partials per batch.

Bias handling (exact):
  - bq, bk folded into the PSUM->SBUF copies of Q^T/K^T (per-partition bias).
  - bk is softmax-invariant but folded anyway (exactness for free).
  - bv, bo: softmax rows sum to one, so  norm(P(V+bv))Wo + bo
    = norm(PV)Wo + (bv Wo + bo); the constant row vector is added on host.

Softmax: scores are ~N(0,1) after the 1/sqrt(PD) scaling (|s| < ~7), so
exp() without the max-subtraction is numerically safe in fp32/bf16 and
mathematically identical to jax.nn.softmax after normalization.
"""

import sys
from contextlib import ExitStack

import numpy as np

sys.path.insert(0, "/opt/trn_rl_repo")

import ml_dtypes  # noqa: E402

import concourse.bass as bass  # noqa: E402
import concourse.mybir as mybir  # noqa: E402
import concourse.tile as tile  # noqa: E402
from concourse import bacc  # noqa: E402
from concourse.bass_utils import run_bass_kernel_spmd  # noqa: E402
from concourse.masks import make_identity  # noqa: E402

B, S, D, H = 4, 4096, 512, 2
PD = D // H          # 256 head dim
P = 128              # partitions
CC = D // P          # 4 contraction chunks over D
DT = PD // P         # 2 partition-tiles over head dim
QB = 512             # q block width (PSUM bank)
NQB = S // QB        # 8
NKT = S // P         # 32 k tiles
F32 = mybir.dt.float32
BF16 = mybir.dt.bfloat16
SCALE = 1.0 / float(np.sqrt(PD))
NCORES = 8
AF = mybir.ActivationFunctionType


def _attention_body(tc, out, xT, wq, wk, wv, wo, bq, bk):
    nc = tc.nc
    NPAIR = NKT // 2  # 16 S^T pairs per q block (exp over 2 PSUM banks)
    with ExitStack() as ctx:
        const = ctx.enter_context(tc.tile_pool(name="const", bufs=1))
        xtp = ctx.enter_context(tc.tile_pool(name="xtp", bufs=CC))
        qk = ctx.enter_context(tc.tile_pool(name="qk", bufs=1))
        vp = ctx.enter_context(tc.tile_pool(name="vp", bufs=1))
        ptp = ctx.enter_context(tc.tile_pool(name="ptp", bufs=34))
        atp = ctx.enter_context(tc.tile_pool(name="atp", bufs=4))
        smal = ctx.enter_context(tc.tile_pool(name="smal", bufs=6))
        outp = ctx.enter_context(tc.tile_pool(name="outp", bufs=4))
        pstp = ctx.enter_context(tc.tile_pool(name="pstp", bufs=2, space="PSUM"))
        psa = ctx.enter_context(tc.tile_pool(name="psa", bufs=3, space="PSUM"))
        pstr = ctx.enter_context(tc.tile_pool(name="pstr", bufs=1, space="PSUM"))

        # constants and weights; x is loaded in (c-chunk x s-half) pieces so
        # the first projection matmuls only wait for the first s-half
        ident = const.tile([P, P], BF16)
        make_identity(nc, ident[:])

        wq_sb = const.tile([P, CC, PD], BF16)
        nc.sync.dma_start(out=wq_sb[:], in_=wq.rearrange("(c p) d -> p c d", p=P))
        wk_sb = const.tile([P, CC, PD], BF16)
        nc.sync.dma_start(out=wk_sb[:], in_=wk.rearrange("(c p) d -> p c d", p=P))
        bq_sb = const.tile([P, DT], F32)
        nc.sync.dma_start(out=bq_sb[:], in_=bq.rearrange("(t p) -> p t", p=P))
        bk_sb = const.tile([P, DT], F32)
        nc.sync.dma_start(out=bk_sb[:], in_=bk.rearrange("(t p) -> p t", p=P))

        HS = S // 2
        xr = xT.rearrange("(c p) s -> c p s", p=P)
        xt_sb = []
        for c in range(CC):
            xc = xtp.tile([P, S], BF16, tag="xt", name=f"xt{c}")
            xt_sb.append(xc)
        for half in range(2):
            for c in range(CC):
                nc.sync.dma_start(
                    out=xt_sb[c][:, half * HS:(half + 1) * HS],
                    in_=xr[c, :, half * HS:(half + 1) * HS],
                )
            if half == 0:
                wv_sb = const.tile([P, CC, PD], BF16)
                nc.sync.dma_start(
                    out=wv_sb[:], in_=wv.rearrange("(c p) d -> p c d", p=P)
                )
                wo_sb = const.tile([P, DT, D], BF16)
                nc.sync.dma_start(
                    out=wo_sb[:], in_=wo.rearrange("(t p) e -> p t e", p=P)
                )

        qt_sb = qk.tile([P, DT, S], BF16)           # Q^T  [d, s]
        kt_sb = qk.tile([P, DT, S], BF16)           # K^T  [d, s]
        v_sb = vp.tile([P, NKT, PD + 1], BF16)      # V    [s, d] + ones col
        nc.vector.memset(v_sb[:, :, PD:PD + 1], 1.0)

        def proj_qk(w_sb, b_sb, dst, dt, sb):
            acc = psa.tile([P, QB], F32, tag="acc", name="acc_p")
            for c in range(CC):
                nc.tensor.matmul(
                    acc[:],
                    w_sb[:, c, dt * P:(dt + 1) * P],
                    xt_sb[c][:, sb * QB:(sb + 1) * QB],
                    start=(c == 0), stop=(c == CC - 1),
                )
            nc.vector.tensor_scalar_add(
                dst[:, dt, sb * QB:(sb + 1) * QB], acc[:], b_sb[:, dt:dt + 1]
            )

        def proj_v(st):
            acc = psa.tile([P, PD], F32, tag="acc", name="acc_v")
            for c in range(CC):
                nc.tensor.matmul(
                    acc[:],
                    xt_sb[c][:, st * P:(st + 1) * P],
                    wv_sb[:, c, :],
                    start=(c == 0), stop=(c == CC - 1),
                )
            nc.vector.tensor_copy(v_sb[:, st, 0:PD], acc[:])

        pt_tiles = {}  # (qb, pair) -> tile [P, 2, QB]

        def st_pair(qb, pair):
            # scores^T for k tiles (2*pair, 2*pair+1), exp over both banks
            acc = pstp.tile([P, 2, QB], F32, tag="st", name="acc_st")
            for par in range(2):
                kt = 2 * pair + par
                for dt in range(DT):
                    nc.tensor.matmul(
                        acc[:, par, :],
                        kt_sb[:, dt, kt * P:(kt + 1) * P],
                        qt_sb[:, dt, qb * QB:(qb + 1) * QB],
                        start=(dt == 0), stop=(dt == DT - 1),
                    )
            ptt = ptp.tile([P, 2, QB], BF16, tag="pt", name="ptt")
            nc.scalar.activation(ptt[:], acc[:], AF.Exp, scale=SCALE)
            pt_tiles[(qb, pair)] = ptt

        # interleaved schedule state
        pend = {}

        def at_step(gs, fn):
            pend.setdefault(gs, []).append(fn)

        def flush(gs):
            for fn in pend.pop(gs, []):
                fn()

        att = {}      # (qb, dt) -> attn^T tile [P, QB]
        attn_n = {}   # (qb, qt) -> normalized attn [P, PD]

        def norm(qb, qt, acc):
            rcp = smal.tile([P, 1], F32, tag="rcp", name="rcp")
            nc.vector.reciprocal(rcp[:], acc[:, PD:PD + 1])
            an = smal.tile([P, PD], BF16, tag="attn_n", name="attn_n")
            nc.vector.tensor_scalar_mul(an[:], acc[:, 0:PD], rcp[:])
            attn_n[(qb, qt)] = an

        def tr(qb, qt):
            an = attn_n.pop((qb, qt))
            trp = pstr.tile([P, DT * P], BF16, tag="tr", name="trp")
            for dt in range(DT):
                nc.tensor.transpose(
                    trp[:, dt * P:(dt + 1) * P], an[:, dt * P:(dt + 1) * P],
                    ident[:],
                )
                nc.vector.tensor_copy(
                    att[(qb, dt)][:, qt * P:(qt + 1) * P],
                    trp[:, dt * P:(dt + 1) * P],
                )

        def o_proj(qb, qt):
            acc = psa.tile([P, D], F32, tag="acc", name="acc_o")
            for dt in range(DT):
                nc.tensor.matmul(
                    acc[:],
                    att[(qb, dt)][:, qt * P:(qt + 1) * P],
                    wo_sb[:, dt, :],
                    start=(dt == 0), stop=(dt == DT - 1),
                )
            osb = outp.tile([P, D], F32, tag="out", name="osb")
            nc.vector.tensor_copy(osb[:], acc[:])
            r0 = qb * QB + qt * P
            nc.sync.dma_start(out=out[r0:r0 + P, :], in_=osb[:])

        # ---- prologue ----
        # s-half 0 units first (their x quarter-DMAs land first), then the
        # half-1 units, with S^T(0) interleaved once all of K is in flight.
        for dt in range(DT):
            proj_qk(wq_sb, bq_sb, qt_sb, dt, 0)
        for sb in range(4):
            for dt in range(DT):
                proj_qk(wk_sb, bk_sb, kt_sb, dt, sb)
        for dt in range(DT):
            for sb in range(1, 4):
                proj_qk(wq_sb, bq_sb, qt_sb, dt, sb)
        for st in range(16):
            proj_v(st)
        for sb in range(4, NQB):
            for dt in range(DT):
                proj_qk(wk_sb, bk_sb, kt_sb, dt, sb)
        rest = (
            [lambda dt=dt, sb=sb: proj_qk(wq_sb, bq_sb, qt_sb, dt, sb)
             for sb in range(4, NQB) for dt in range(DT)]
            + [lambda st=st: proj_v(st) for st in range(16, NKT)]
        )
        for p in range(NPAIR):
            st_pair(0, p)
            for _ in range(2 if p % 2 == 0 else 1):
                if rest:
                    rest.pop(0)()
        for fn in rest:
            fn()

        # ---- main loop: interleave S^T(qb+1) with PV/norm/TR/O of qb ----
        for qb in range(NQB):
            for d in range(DT):
                att[(qb, d)] = atp.tile([P, QB], BF16, tag=f"at{d}",
                                        name=f"att{d}")
            for step in range(32):
                gs = qb * 32 + step
                qt, j = divmod(step, 8)
                if qb + 1 < NQB and step % 2 == 0:
                    st_pair(qb + 1, step // 2)
                if j == 0:
                    acc_pv = psa.tile([P, PD + 1], F32, tag="acc",
                                      name="acc_pv")
                for m in range(4):
                    kt = j * 4 + m
                    pair, par = divmod(kt, 2)
                    nc.tensor.matmul(
                        acc_pv[:],
                        pt_tiles[(qb, pair)][:, par, qt * P:(qt + 1) * P],
                        v_sb[:, kt, :],
                        start=(kt == 0), stop=(kt == NKT - 1),
                    )
                if j == 7:
                    norm(qb, qt, acc_pv)
                    at_step(gs + 2, lambda qb=qb, qt=qt: tr(qb, qt))
                    at_step(gs + 4, lambda qb=qb, qt=qt: o_proj(qb, qt))
                flush(gs)
            # drop references to consumed P^T tiles of this qb
            for pair in range(NPAIR):
                pt_tiles.pop((qb, pair), None)

        # tail: flush any remaining deferred work (TR/O of the last q tiles)
        for gs in sorted(pend):
            for fn in pend.pop(gs, []):
                fn()


_NC_CACHE = None


def _build_nc():
    global _NC_CACHE
    if _NC_CACHE is not None:
        return _NC_CACHE
    nc = bacc.Bacc(
        "TRN2", target_bir_lowering=False, debug=False, num_devices=NCORES
    )
    xT = nc.dram_tensor("xT", [D, S], BF16, kind="ExternalInput").ap()
    wq = nc.dram_tensor("wq", [D, PD], BF16, kind="ExternalInput").ap()
    wk = nc.dram_tensor("wk", [D, PD], BF16, kind="ExternalInput").ap()
    wv = nc.dram_tensor("wv", [D, PD], BF16, kind="ExternalInput").ap()
    wo = nc.dram_tensor("wo", [PD, D], BF16, kind="ExternalInput").ap()
    bq = nc.dram_tensor("bq", [PD], F32, kind="ExternalInput").ap()
    bk = nc.dram_tensor("bk", [PD], F32, kind="ExternalInput").ap()
    out = nc.dram_tensor("out", [S, D], F32, kind="ExternalOutput").ap()
    with tile.TileContext(nc) as tc:
        _attention_body(tc, out, xT, wq, wk, wv, wo, bq, bk)
    nc.compile()
    _NC_CACHE = nc
    return nc


def _run(inputs, **spmd_kwargs):
    x = np.asarray(inputs["x"], np.float32)
    Wq = np.asarray(inputs["Wq"], np.float32)
    Wk = np.asarray(inputs["Wk"], np.float32)
    Wv = np.asarray(inputs["Wv"], np.float32)
    Wo = np.asarray(inputs["Wo"], np.float32)
    bq = np.asarray(inputs["bq"], np.float32)
    bk = np.asarray(inputs["bk"], np.float32)
    bv = np.asarray(inputs["bv"], np.float32)
    bo = np.asarray(inputs["bo"], np.float32)

    bf = ml_dtypes.bfloat16
    xT = [np.ascontiguousarray(x[b].T).astype(bf) for b in range(B)]
    in_maps = []
    for core in range(NCORES):
        b, h = divmod(core, H)
        hs = slice(h * PD, (h + 1) * PD)
        in_maps.append({
            "xT": xT[b],
            "wq": np.ascontiguousarray(Wq[:, hs]).astype(bf),
            "wk": np.ascontiguousarray(Wk[:, hs]).astype(bf),
            "wv": np.ascontiguousarray(Wv[:, hs]).astype(bf),
            "wo": np.ascontiguousarray(Wo[hs, :]).astype(bf),
            "bq": np.ascontiguousarray(bq[hs]),
            "bk": np.ascontiguousarray(bk[hs]),
        })

    nc = _build_nc()
    res = run_bass_kernel_spmd(nc, in_maps, list(range(NCORES)), **spmd_kwargs)

    out = np.zeros((B, S, D), np.float32)
    for core in range(NCORES):
        b = core // H
        out[b] += res.results[core]["out"]
    out += bv @ Wo + bo  # exact bias correction (softmax rows sum to 1)
    return out, res


def kernel(**inputs):
    out, _ = _run(inputs)
    return out


# revision 9
# speedup vs baseline: 1.1092x; 1.0007x over previous
"""Multi-head attention (B=4, S=4096, D=512, H=2) on 8 TRN2 NeuronCores.

Sharding: one (batch, head) pair per core -> 8 cores, perfectly balanced,
no collectives. Host pre-transposes x per batch to x^T (bf16) and slices
the weights per head; device computes the full attention for its pair and
the partial output projection; host sums the two head partials per batch.

Bias handling (exact):
  - bq, bk folded into the PSUM->SBUF copies of Q^T/K^T (per-partition bias).
  - bk is softmax-invariant but folded anyway (exactness for free).
  - bv, bo: softmax rows sum to one, so  norm(P(V+bv))Wo + bo
    = norm(PV)Wo + (bv Wo + bo); the constant row vector is added on host.

Softmax: scores are ~N(0,1) after the 1/sqrt(PD) scaling (|s| < ~7), so
exp() without the max-subtraction is numerically safe in fp32/bf16 and
mathematically identical to jax.nn.softmax after normalization.
"""

import sys
from contextlib import ExitStack

import numpy as np

sys.path.insert(0, "/opt/trn_rl_repo")

import ml_dtypes  # noqa: E402

import concourse.bass as bass  # noqa: E402
import concourse.mybir as mybir  # noqa: E402
import concourse.tile as tile  # noqa: E402
from concourse import bacc  # noqa: E402
from concourse.bass_utils import run_bass_kernel_spmd  # noqa: E402
from concourse.masks import make_identity  # noqa: E402

B, S, D, H = 4, 4096, 512, 2
PD = D // H          # 256 head dim
P = 128              # partitions
CC = D // P          # 4 contraction chunks over D
DT = PD // P         # 2 partition-tiles over head dim
QB = 512             # q block width (PSUM bank)
NQB = S // QB        # 8
NKT = S // P         # 32 k tiles
F32 = mybir.dt.float32
BF16 = mybir.dt.bfloat16
SCALE = 1.0 / float(np.sqrt(PD))
NCORES = 8
AF = mybir.ActivationFunctionType


def _attention_body(tc, out, xT, wq, wk, wv, wo, bq, bk):
    nc = tc.nc
    NPAIR = NKT // 2  # 16 S^T pairs per q block (exp over 2 PSUM banks)
    with ExitStack() as ctx:
        const = ctx.enter_context(tc.tile_pool(name="const", bufs=1))
        xtp = ctx.enter_context(tc.tile_pool(name="xtp", bufs=CC))
        qk = ctx.enter_context(tc.tile_pool(name="qk", bufs=1))
        vp = ctx.enter_context(tc.tile_pool(name="vp", bufs=1))
        ptp = ctx.enter_context(tc.tile_pool(name="ptp", bufs=34))
        atp = ctx.enter_context(tc.tile_pool(name="atp", bufs=4))
        smal = ctx.enter_context(tc.tile_pool(name="smal", bufs=6))
        outp = ctx.enter_context(tc.tile_pool(name="outp", bufs=4))
        pstp = ctx.enter_context(tc.tile_pool(name="pstp", bufs=2, space="PSUM"))
        psa = ctx.enter_context(tc.tile_pool(name="psa", bufs=3, space="PSUM"))
        pstr = ctx.enter_context(tc.tile_pool(name="pstr", bufs=1, space="PSUM"))

        # constants and weights; x is loaded in (c-chunk x s-half) pieces so
        # the first projection matmuls only wait for the first s-half
        ident = const.tile([P, P], BF16)
        make_identity(nc, ident[:])

        wq_sb = const.tile([P, CC, PD], BF16)
        nc.sync.dma_start(out=wq_sb[:], in_=wq.rearrange("(c p) d -> p c d", p=P))
        bq_sb = const.tile([P, DT], F32)
        nc.sync.dma_start(out=bq_sb[:], in_=bq.rearrange("(t p) -> p t", p=P))
        wk_sb = const.tile([P, CC, PD], BF16)
        nc.sync.dma_start(out=wk_sb[:], in_=wk.rearrange("(c p) d -> p c d", p=P))
        bk_sb = const.tile([P, DT], F32)
        nc.sync.dma_start(out=bk_sb[:], in_=bk.rearrange("(t p) -> p t", p=P))

        xr = xT.rearrange("(c p) s -> c p s", p=P)
        xt_sb = []
        for c in range(CC):
            xc = xtp.tile([P, S], BF16, tag="xt", name=f"xt{c}")
            xt_sb.append(xc)
        # x pieces ordered so the earliest projection units unblock first
        pieces = [(0, QB), (QB, S // 2), (S // 2, S)]
        for pi, (s0, s1) in enumerate(pieces):
            for c in range(CC):
                nc.sync.dma_start(
                    out=xt_sb[c][:, s0:s1], in_=xr[c, :, s0:s1]
                )
            if pi == 0:
                wv_sb = const.tile([P, CC, PD], BF16)
                nc.sync.dma_start(
                    out=wv_sb[:], in_=wv.rearrange("(c p) d -> p c d", p=P)
                )
            elif pi == 1:
                wo_sb = const.tile([P, DT, D], BF16)
                nc.sync.dma_start(
                    out=wo_sb[:], in_=wo.rearrange("(t p) e -> p t e", p=P)
                )

        qt_sb = qk.tile([P, DT, S], BF16)           # Q^T  [d, s]
        kt_sb = qk.tile([P, DT, S], BF16)           # K^T  [d, s]
        v_sb = vp.tile([P, NKT, PD + 1], BF16)      # V    [s, d] + ones col
        nc.vector.memset(v_sb[:, :, PD:PD + 1], 1.0)

        def proj_qk(w_sb, b_sb, dst, dt, sb):
            acc = psa.tile([P, QB], F32, tag="acc", name="acc_p")
            for c in range(CC):
                nc.tensor.matmul(
                    acc[:],
                    w_sb[:, c, dt * P:(dt + 1) * P],
                    xt_sb[c][:, sb * QB:(sb + 1) * QB],
                    start=(c == 0), stop=(c == CC - 1),
                )
            nc.vector.tensor_scalar_add(
                dst[:, dt, sb * QB:(sb + 1) * QB], acc[:], b_sb[:, dt:dt + 1]
            )

        def proj_v(st):
            acc = psa.tile([P, PD], F32, tag="acc", name="acc_v")
            for c in range(CC):
                nc.tensor.matmul(
                    acc[:],
                    xt_sb[c][:, st * P:(st + 1) * P],
                    wv_sb[:, c, :],
                    start=(c == 0), stop=(c == CC - 1),
                )
            nc.vector.tensor_copy(v_sb[:, st, 0:PD], acc[:])

        pt_tiles = {}  # (qb, pair) -> tile [P, 2, QB]

        def st_pair(qb, pair):
            # scores^T for k tiles (2*pair, 2*pair+1), exp over both banks
            acc = pstp.tile([P, 2, QB], F32, tag="st", name="acc_st")
            for par in range(2):
                kt = 2 * pair + par
                for dt in range(DT):
                    nc.tensor.matmul(
                        acc[:, par, :],
                        kt_sb[:, dt, kt * P:(kt + 1) * P],
                        qt_sb[:, dt, qb * QB:(qb + 1) * QB],
                        start=(dt == 0), stop=(dt == DT - 1),
                    )
            ptt = ptp.tile([P, 2, QB], BF16, tag="pt", name="ptt")
            nc.scalar.activation(ptt[:], acc[:], AF.Exp, scale=SCALE)
            pt_tiles[(qb, pair)] = ptt

        # interleaved schedule state
        pend = {}

        def at_step(gs, fn):
            pend.setdefault(gs, []).append(fn)

        def flush(gs):
            for fn in pend.pop(gs, []):
                fn()

        att = {}      # (qb, dt) -> attn^T tile [P, QB]
        attn_n = {}   # (qb, qt) -> normalized attn [P, PD]

        def norm(qb, qt, acc):
            rcp = smal.tile([P, 1], F32, tag="rcp", name="rcp")
            nc.vector.reciprocal(rcp[:], acc[:, PD:PD + 1])
            an = smal.tile([P, PD], BF16, tag="attn_n", name="attn_n")
            nc.vector.tensor_scalar_mul(an[:], acc[:, 0:PD], rcp[:])
            attn_n[(qb, qt)] = an

        def tr(qb, qt):
            an = attn_n.pop((qb, qt))
            trp = pstr.tile([P, DT * P], BF16, tag="tr", name="trp")
            for dt in range(DT):
                nc.tensor.transpose(
                    trp[:, dt * P:(dt + 1) * P], an[:, dt * P:(dt + 1) * P],
                    ident[:],
                )
                nc.vector.tensor_copy(
                    att[(qb, dt)][:, qt * P:(qt + 1) * P],
                    trp[:, dt * P:(dt + 1) * P],
                )

        def o_proj(qb, qt):
            acc = psa.tile([P, D], F32, tag="acc", name="acc_o")
            for dt in range(DT):
                nc.tensor.matmul(
                    acc[:],
                    att[(qb, dt)][:, qt * P:(qt + 1) * P],
                    wo_sb[:, dt, :],
                    start=(dt == 0), stop=(dt == DT - 1),
                )
            osb = outp.tile([P, D], F32, tag="out", name="osb")
            nc.vector.tensor_copy(osb[:], acc[:])
            r0 = qb * QB + qt * P
            nc.sync.dma_start(out=out[r0:r0 + P, :], in_=osb[:])

        # ---- prologue ----
        # s-half 0 units first (their x quarter-DMAs land first), then the
        # half-1 units, with S^T(0) interleaved once all of K is in flight.
        for dt in range(DT):
            proj_qk(wq_sb, bq_sb, qt_sb, dt, 0)
        for sb in range(4):
            for dt in range(DT):
                proj_qk(wk_sb, bk_sb, kt_sb, dt, sb)
        for dt in range(DT):
            for sb in range(1, 4):
                proj_qk(wq_sb, bq_sb, qt_sb, dt, sb)
        for st in range(16):
            proj_v(st)
        for sb in range(4, NQB):
            for dt in range(DT):
                proj_qk(wk_sb, bk_sb, kt_sb, dt, sb)
        rest = (
            [lambda dt=dt, sb=sb: proj_qk(wq_sb, bq_sb, qt_sb, dt, sb)
             for sb in range(4, NQB) for dt in range(DT)]
            + [lambda st=st: proj_v(st) for st in range(16, NKT)]
        )
        for p in range(NPAIR):
            st_pair(0, p)
            for _ in range(2 if p % 2 == 0 else 1):
                if rest:
                    rest.pop(0)()
        for fn in rest:
            fn()

        # ---- main loop: interleave S^T(qb+1) with PV/norm/TR/O of qb ----
        for qb in range(NQB):
            for d in range(DT):
                att[(qb, d)] = atp.tile([P, QB], BF16, tag=f"at{d}",
                                        name=f"att{d}")
            for step in range(32):
                gs = qb * 32 + step
                qt, j = divmod(step, 8)
                if qb + 1 < NQB and step % 2 == 0:
                    st_pair(qb + 1, step // 2)
                if j == 0:
                    acc_pv = psa.tile([P, PD + 1], F32, tag="acc",
                                      name="acc_pv")
                for m in range(4):
                    kt = j * 4 + m
                    pair, par = divmod(kt, 2)
                    nc.tensor.matmul(
                        acc_pv[:],
                        pt_tiles[(qb, pair)][:, par, qt * P:(qt + 1) * P],
                        v_sb[:, kt, :],
                        start=(kt == 0), stop=(kt == NKT - 1),
                    )
                if j == 7:
                    norm(qb, qt, acc_pv)
                    at_step(gs + 2, lambda qb=qb, qt=qt: tr(qb, qt))
                    at_step(gs + 4, lambda qb=qb, qt=qt: o_proj(qb, qt))
                flush(gs)
            # drop references to consumed P^T tiles of this qb
            for pair in range(NPAIR):
                pt_tiles.pop((qb, pair), None)

        # tail: flush any remaining deferred work (TR/O of the last q tiles)
        for gs in sorted(pend):
            for fn in pend.pop(gs, []):
                fn()


_NC_CACHE = None


def _build_nc():
    global _NC_CACHE
    if _NC_CACHE is not None:
        return _NC_CACHE
    nc = bacc.Bacc(
        "TRN2", target_bir_lowering=False, debug=False, num_devices=NCORES
    )
    xT = nc.dram_tensor("xT", [D, S], BF16, kind="ExternalInput").ap()
    wq = nc.dram_tensor("wq", [D, PD], BF16, kind="ExternalInput").ap()
    wk = nc.dram_tensor("wk", [D, PD], BF16, kind="ExternalInput").ap()
    wv = nc.dram_tensor("wv", [D, PD], BF16, kind="ExternalInput").ap()
    wo = nc.dram_tensor("wo", [PD, D], BF16, kind="ExternalInput").ap()
    bq = nc.dram_tensor("bq", [PD], F32, kind="ExternalInput").ap()
    bk = nc.dram_tensor("bk", [PD], F32, kind="ExternalInput").ap()
    out = nc.dram_tensor("out", [S, D], F32, kind="ExternalOutput").ap()
    with tile.TileContext(nc) as tc:
        _attention_body(tc, out, xT, wq, wk, wv, wo, bq, bk)
    nc.compile()
    _NC_CACHE = nc
    return nc


def _run(inputs, **spmd_kwargs):
    x = np.asarray(inputs["x"], np.float32)
    Wq = np.asarray(inputs["Wq"], np.float32)
    Wk = np.asarray(inputs["Wk"], np.float32)
    Wv = np.asarray(inputs["Wv"], np.float32)
    Wo = np.asarray(inputs["Wo"], np.float32)
    bq = np.asarray(inputs["bq"], np.float32)
    bk = np.asarray(inputs["bk"], np.float32)
    bv = np.asarray(inputs["bv"], np.float32)
    bo = np.asarray(inputs["bo"], np.float32)

    bf = ml_dtypes.bfloat16
    xT = [np.ascontiguousarray(x[b].T).astype(bf) for b in range(B)]
    in_maps = []
    for core in range(NCORES):
        b, h = divmod(core, H)
        hs = slice(h * PD, (h + 1) * PD)
        in_maps.append({
            "xT": xT[b],
            "wq": np.ascontiguousarray(Wq[:, hs]).astype(bf),
            "wk": np.ascontiguousarray(Wk[:, hs]).astype(bf),
            "wv": np.ascontiguousarray(Wv[:, hs]).astype(bf),
            "wo": np.ascontiguousarray(Wo[hs, :]).astype(bf),
            "bq": np.ascontiguousarray(bq[hs]),
            "bk": np.ascontiguousarray(bk[hs]),
        })

    nc = _build_nc()
    res = run_bass_kernel_spmd(nc, in_maps, list(range(NCORES)), **spmd_kwargs)

    out = np.zeros((B, S, D), np.float32)
    for core in range(NCORES):
        b = core // H
        out[b] += res.results[core]["out"]
    out += bv @ Wo + bo  # exact bias correction (softmax rows sum to 1)
    return out, res


def kernel(**inputs):
    out, _ = _run(inputs)
    return out
